# revision 33
# baseline (speedup 1.0000x reference)
"""CX loss kernel for Trainium2 (8 NeuronCores, SPMD).

Math (identical to the reference):
  d[q,p]  = normalize(fI[q]-m) . normalize(fT[p]-m),  m = mean of fT over N,H,W
  CX[q,p] = softmax_p(kappa_q * d[q,p]),  kappa_q = 10 / (1 - max_p d + 2*EPS)
  T[p]    = max_q CX[q,p];  loss = mean_n(-log(mean_p T))

Split of work:
  DEVICE (8 cores = 2 batches x 4 query blocks of 1024): the O(N*P^2*C)
  correlation d = A^T W, twice per 128-query tile:
    pass A (fp8 DoubleRow: full C=256 contraction in one matmul, half-rate
    columns) fills PSUM quarters that the DVE row-max scan consumes -- this
    yields the per-query softmax temperature kappa;
    pass B (fp8 DoubleRow by default, bf16 with CX_B8=0) recomputes d, and
    ACT exps it straight out of PSUM with kappa folded into the activation
    scale.
  Each tile's exp table (bf16) streams straight to DRAM over the
  otherwise-idle DMA engines; passes A and B own separate PSUM
  quarter-buffers so the A lane runs ahead of the B lane.
  HOST (cheap): input prep (center by meanT, L2-normalize, quantize) and the
  final fold: CX = et/sum(et), max over query tiles / lanes / cores, mean,
  -log.
"""

import os
import sys
import numpy as np

if "/opt/trn_rl_repo" not in sys.path:
    sys.path.insert(0, "/opt/trn_rl_repo")

N, C, H, Wd = 2, 256, 64, 64
P = H * Wd            # 4096 target patches / queries per batch
QB = P // 4           # 1024 queries per core
EPS = 1e-5
NCORES = 8
HP = P // 2           # 2048
QW = P // 4           # 1024 cols per PSUM quarter
NT = 8                # query tiles per core
F8SCALE = 8.0         # fp8 operand scale; z8 = F8SCALE^2 * z

_CACHE = {}

N_WARM_MM = int(os.environ.get("CX_WARM", "0"))
B_FP8 = os.environ.get("CX_B8", "1") == "1"
# tensor_tensor_reduce with op1=max faults at runtime on real HW
USE_TTR = os.environ.get("CX_TTR", "0") == "1"


def _build():
    import concourse.bacc as bacc
    import concourse.bass as bass
    import concourse.mybir as mybir
    import concourse.tile as tile

    f32 = mybir.dt.float32
    bf16 = mybir.dt.bfloat16
    fp8 = mybir.dt.float8e4
    AX = mybir.AxisListType.X
    OP = mybir.AluOpType
    AF = mybir.ActivationFunctionType
    DR = mybir.MatmulPerfMode.DoubleRow

    nc = bacc.Bacc("TRN2", target_bir_lowering=False, debug=False,
                   num_devices=NCORES)

    a8_d = nc.dram_tensor("A8", [128, 2, QB], fp8, kind="ExternalInput")
    w8_d = nc.dram_tensor("W8", [128, 2, P], fp8, kind="ExternalInput")
    if not B_FP8:
        a_d = nc.dram_tensor("A", [C, QB], bf16, kind="ExternalInput")
        w_d = nc.dram_tensor("W", [C, P], bf16, kind="ExternalInput")
    et_d = nc.dram_tensor("ET", [NT, 128, P], bf16, kind="ExternalOutput")

    def T(pool, shape, dtype, tag):
        return pool.tile(shape, dtype, tag=tag, name=tag)

    with tile.TileContext(nc) as tc:
        with (
            tc.tile_pool(name="big", bufs=1) as big,
            tc.tile_pool(name="sm", bufs=1) as sm,
            tc.tile_pool(name="loop", bufs=3) as lp,
            tc.tile_pool(name="ps", bufs=1,
                         space=bass.MemorySpace.PSUM) as ps,
        ):
            # pass A / pass B PSUM quarter-buffers (4 x 2 banks = 8 banks)
            za = [T(ps, [128, QW], f32, f"za{i}") for i in range(2)]
            zb = [T(ps, [128, QW], f32, f"zb{i}") for i in range(2)]

            # preload the exp ACT table at t=0 (overlaps the input DMAs)
            dummy = T(sm, [1, 1], f32, "dummy")
            nc.vector.memset(dummy[:], 1.0)
            dummy2 = T(sm, [1, 1], f32, "dummy2")
            nc.scalar.activation(dummy2[:], dummy[:], AF.Exp)

            # PE clock warm-up on zeros while the input DMAs stream
            # (irrelevant when DVE is the bottleneck; default off)
            if N_WARM_MM:
                wlhs = T(sm, [128, 128], bf16, "wlhs")
                nc.vector.memset(wlhs[:], 0.0)
                wrhs = T(sm, [128, 512], bf16, "wrhs")
                nc.vector.memset(wrhs[:], 0.0)
                for i in range(N_WARM_MM):
                    nc.tensor.matmul(za[0][:, 0:512], wlhs[:], wrhs[:],
                                     start=True, stop=True)

            a8_sb = T(big, [128, 2, QB], fp8, "a8")
            w8_sb = T(big, [128, 2, P], fp8, "w8")

            # fp8 operands; w8 split so tile 0's pass A starts early
            nc.sync.dma_start(a8_sb[:], a8_d.ap())
            for q in range(4):
                nc.sync.dma_start(w8_sb[:, :, QW * q:QW * (q + 1)],
                                  w8_d.ap()[:, :, QW * q:QW * (q + 1)])
            if not B_FP8:
                a_sb = [T(big, [128, QB], bf16, f"a{k}") for k in range(2)]
                w_sb = [T(big, [128, P], bf16, f"w{k}") for k in range(2)]
                for k in range(2):
                    nc.sync.dma_start(a_sb[k][:],
                                      a_d.ap()[128 * k:128 * (k + 1), :])
                for k in range(2):
                    nc.sync.dma_start(w_sb[k][:, 0:HP],
                                      w_d.ap()[128 * k:128 * (k + 1), 0:HP])
                for k in range(2):
                    nc.sync.dma_start(w_sb[k][:, HP:P],
                                      w_d.ap()[128 * k:128 * (k + 1), HP:P])

            def mm_8(dst, q, t):
                # fp8 DoubleRow: lhsT [128,2,128] x rhs [128,2,512]
                # contracts all 256 channels in one half-rate matmul
                qs = slice(128 * t, 128 * (t + 1))
                for c2 in range(2):
                    cols = slice(512 * c2, 512 * (c2 + 1))
                    wcols = slice(QW * q + 512 * c2, QW * q + 512 * (c2 + 1))
                    nc.tensor.matmul(dst[:, cols],
                                     a8_sb[:, :, qs],
                                     w8_sb[:, :, wcols],
                                     start=True, stop=True, perf_mode=DR)

            def mm_bf(dst, q, t):
                qs = slice(128 * t, 128 * (t + 1))
                for c2 in range(2):
                    cols = slice(512 * c2, 512 * (c2 + 1))
                    wcols = slice(QW * q + 512 * c2, QW * q + 512 * (c2 + 1))
                    for k in range(2):
                        nc.tensor.matmul(dst[:, cols],
                                         a_sb[k][:, qs],
                                         w_sb[k][:, wcols],
                                         start=(k == 0), stop=(k == 1))

            # z8 = SS*z, so the exp scale is kappa/b_scale:
            # kappa/b = 1/(b*0.1*(1+2eps) - (b*0.1/SS)*mx8)
            SS = F8SCALE * F8SCALE
            b_scale = SS if B_FP8 else 1.0

            if USE_TTR:
                neginf = T(sm, [128, QW], f32, "neginf")
                nc.vector.memset(neginf[:], -3.0e38)

            for t in range(NT):
                # pass A: fill z quarters (fp8).  Row-max scan via DVE
                # tensor_tensor_reduce (only ONE input may live in PSUM, so
                # the second input is a constant -inf SBUF tile); each scan
                # chains the previous accumulator as its initial value so no
                # separate combine op is needed.
                if USE_TTR:
                    junk = T(lp, [128, QW], bf16, "junk")
                mxs = T(lp, [128, 4], f32, "mxs")
                mxf = T(lp, [128, 1], f32, "mxf")
                for q in range(4):
                    mm_8(za[q % 2], q, t)
                    if USE_TTR:
                        nc.vector.tensor_tensor_reduce(
                            junk[:], za[q % 2][:], neginf[:], 1.0,
                            -3.0e38 if q == 0 else mxs[:, q - 1:q],
                            op0=OP.max, op1=OP.max,
                            accum_out=mxs[:, q:q + 1])
                    else:
                        nc.vector.reduce_max(mxs[:, q:q + 1], za[q % 2][:],
                                             axis=AX)
                if not USE_TTR:
                    nc.vector.reduce_max(mxf[:], mxs[:], axis=AX)
                den = T(lp, [128, 1], f32, "den")
                nc.vector.tensor_scalar(den[:],
                                        mxs[:, 3:4] if USE_TTR else mxf[:],
                                        -0.1 * b_scale / SS,
                                        0.1 * b_scale * (1.0 + 2.0 * EPS),
                                        op0=OP.mult, op1=OP.add)
                kap = T(lp, [128, 1], f32, "kap")
                nc.vector.reciprocal(kap[:], den[:])
                # pass B: recompute z, exp straight out of PSUM with the
                # temperature folded into ACT's scale; row-sums via accum.
                # et halves + row-sums stream to DRAM on the idle DMA rails.
                et = T(lp, [128, P], bf16, "et")
                for q in range(4):
                    if B_FP8:
                        mm_8(zb[q % 2], q, t)
                    else:
                        mm_bf(zb[q % 2], q, t)
                    nc.scalar.activation(et[:, QW * q:QW * (q + 1)],
                                         zb[q % 2][:], AF.Exp,
                                         scale=kap[:, 0:1])
                    if t == NT - 1:
                        cols = slice(QW * q, QW * (q + 1))
                        nc.sync.dma_start(et_d.ap()[t, :, cols], et[:, cols])
                    elif q % 2 == 1:
                        cols = slice(HP * (q // 2), HP * (q // 2 + 1))
                        nc.sync.dma_start(et_d.ap()[t, :, cols], et[:, cols])

    nc.compile()
    return nc


def _get_nc():
    if "nc" not in _CACHE:
        _CACHE["nc"] = _build()
    return _CACHE["nc"]


def _prep(featureT, featureI):
    """Host-side prep: center by meanT, L2-normalize channels, quantize."""
    import ml_dtypes

    fT = np.asarray(featureT, dtype=np.float32).reshape(N, C, P)
    fI = np.asarray(featureI, dtype=np.float32).reshape(N, C, P)
    m = fT.mean(axis=(0, 2))[None, :, None]          # [1, C, 1]
    fTc = fT - m
    fIc = fI - m
    wn = fTc / np.sqrt((fTc * fTc).sum(axis=1, keepdims=True))
    an = fIc / np.sqrt((fIc * fIc).sum(axis=1, keepdims=True))
    w8 = (wn * F8SCALE).astype(ml_dtypes.float8_e4m3)
    a8 = (an * F8SCALE).astype(ml_dtypes.float8_e4m3)
    # DoubleRow layout: [c, j, x] = X8[c + 128*j, x]
    w8 = np.ascontiguousarray(w8.reshape(N, 2, 128, P).transpose(0, 2, 1, 3))
    a8 = np.ascontiguousarray(a8.reshape(N, 2, 128, P).transpose(0, 2, 1, 3))
    in_maps = []
    for core in range(NCORES):
        n = core // 4
        qb = core % 4
        qsl = slice(qb * QB, (qb + 1) * QB)
        im = {
            "A8": np.ascontiguousarray(a8[n][:, :, qsl]),
            "W8": np.ascontiguousarray(w8[n]),
        }
        if not B_FP8:
            wb = wn.astype(ml_dtypes.bfloat16)
            ab = an.astype(ml_dtypes.bfloat16)
            im["A"] = np.ascontiguousarray(ab[n][:, qsl])
            im["W"] = np.ascontiguousarray(wb[n])
        in_maps.append(im)
    return in_maps


def _run(featureT, featureI, trace=False):
    from concourse.bass_utils import run_bass_kernel_spmd

    nc = _get_nc()
    in_maps = _prep(featureT, featureI)
    res = run_bass_kernel_spmd(nc, in_maps, list(range(NCORES)), trace=trace)
    return res


def _finish(results):
    loss = 0.0
    for n in range(N):
        t_n = None
        for core in range(4 * n, 4 * n + 4):
            et = results[core]["ET"].astype(np.float32).reshape(NT, 128, P)
            r = 1.0 / et.sum(axis=2)                     # [NT, 128]
            tv = (et * r[:, :, None]).max(axis=(0, 1))   # [P]
            t_n = tv if t_n is None else np.maximum(t_n, tv)
        loss += -np.log(np.mean(t_n.astype(np.float64)))
    return np.float32(loss / N)


def kernel(featureT, featureI):
    res = _run(featureT, featureI, trace=False)
    return _finish(res.results)
